# revision 12
# baseline (speedup 1.0000x reference)
"""Trainium2 Bass kernel: grouped-experts SwiGLU MLP with mid-RMSNorm.

Expert-parallel across 8 NeuronCores: core e computes expert e's token
block (tokens are pre-sorted by expert).  Host gathers each expert's
rows into a zero-padded [C, D] buffer, ships transposed activations and
weights, and scatters the per-core outputs back to flat token order.

Per-core math (fp16 operands, fp32 PSUM accumulation):
    h1 = x @ w1^T ; h3 = x @ w3^T          # [C, F]
    h  = silu(h1) * h3
    h  = h * rsqrt(mean(h^2) + eps)        # RMSNorm (scale folded to out)
    out = (h * mid_w) @ w2^T               # mid_w folded into w2 on host

Schedule (all weights SBUF-resident, streamed in once):
  fb0 w1-sweep k-outer (paces the initial weight/x DMAs), then t-outer
  sweeps for fb0 w3, fb1 w1, fb1 w3.  h tiles are transposed via the
  DMA xbar (dma_start_transpose) so the PE only runs the 480 matmuls.
  Phase C (out = hT.T @ w2T, scaled by rstd) runs per-tile at the end.
"""

import sys

sys.path.insert(0, "/opt/trn_rl_repo")

import numpy as np
from contextlib import ExitStack

import os

import concourse.bass as bass
import concourse.tile as tile
from concourse import bacc, mybir

P = 128
T = 4096
D = 2048
F = 1024
E = 8
NB = 512  # matmul moving-dim block (one PSUM bank of fp32)
EPS = 1e-6
F32 = mybir.dt.float32
F16 = mybir.dt.float16
ACTF = mybir.ActivationFunctionType

_PROGRAM_CACHE: dict[int, object] = {}
LAST_RESULTS = None  # test harness reads per-core outputs from here


def _run(nc, in_maps):
    """Execute the compiled program on the 8 axon-tunneled cores.

    If KERNEL_NTFF_DIR is set, wrap the execute in the axon NTFF profile
    hook so device profiles land there (test harness use only).
    """
    from concourse import bass2jax

    ntff_dir = os.environ.get("KERNEL_NTFF_DIR")
    if ntff_dir:
        if "/root/.axon_site" not in sys.path:
            sys.path.insert(0, "/root/.axon_site")
        from trn_agent_boot.trn_boot import _ntff_profile_via_ctypes

        hook = _ntff_profile_via_ctypes("/opt/axon/libaxon_pjrt.so")
        ids = [
            int(x) for x in os.environ.get("KERNEL_NTFF_CORES", "0").split(",")
        ]
        if hook is not None:
            with hook(ntff_dir, ids):
                return bass2jax.run_bass_via_pjrt(nc, in_maps, n_cores=len(in_maps))
    return bass2jax.run_bass_via_pjrt(nc, in_maps, n_cores=len(in_maps))


def _build_program(C: int):
    """Build + compile the single-core SPMD program for C padded rows."""
    NT = C // P  # token tiles per core (<= 6: PSUM holds NT + 2 banks)
    KD = D // P  # 16 contraction chunks for mm1
    KF = F // P  # 8 contraction chunks for mm2
    FB = F // NB  # 2 f-blocks
    DB = D // NB  # 4 d-blocks

    nc = bacc.Bacc(
        "TRN2",
        target_bir_lowering=False,
        debug=False,
        enable_asserts=False,
        num_devices=E,
    )
    xT_d = nc.dram_tensor("xT", [D, C], F16, kind="ExternalInput").ap()
    w1_d = nc.dram_tensor("w1t", [D, F], F16, kind="ExternalInput").ap()
    w3_d = nc.dram_tensor("w3t", [D, F], F16, kind="ExternalInput").ap()
    w2_d = nc.dram_tensor("w2t", [F, D], F16, kind="ExternalInput").ap()
    out_d = nc.dram_tensor("out", [C, D], F16, kind="ExternalOutput").ap()

    with tile.TileContext(nc) as tc, ExitStack() as ctx:
        singles = ctx.enter_context(tc.tile_pool(name="singles", bufs=1))
        xpool = ctx.enter_context(tc.tile_pool(name="x", bufs=1))
        w1pool = ctx.enter_context(tc.tile_pool(name="w1", bufs=1))
        w3pool = ctx.enter_context(tc.tile_pool(name="w3", bufs=1))
        w2pool = ctx.enter_context(tc.tile_pool(name="w2", bufs=1))
        hpool = ctx.enter_context(tc.tile_pool(name="h", bufs=1))
        htpool = ctx.enter_context(tc.tile_pool(name="ht", bufs=1))
        spool = ctx.enter_context(tc.tile_pool(name="s", bufs=1))
        qpool = ctx.enter_context(tc.tile_pool(name="sq", bufs=2))
        opool = ctx.enter_context(tc.tile_pool(name="o", bufs=2))
        stat = ctx.enter_context(tc.tile_pool(name="stat", bufs=1))
        ps_a = ctx.enter_context(tc.tile_pool(name="psa", bufs=1, space="PSUM"))
        ps_o = ctx.enter_context(tc.tile_pool(name="pso", bufs=2, space="PSUM"))

        eps_t = singles.tile([P, 1], F32, name="epsT")
        nc.gpsimd.memset(eps_t[:], EPS)

        # ---- input DMAs, issued upfront but paced with scheduler time
        # floors so the instantaneous HBM rate stays low: saturating DMA
        # roughly halves the PE matmul issue rate (SBUF port contention),
        # so every stream arrives just-in-time for its sweep.
        # x + w2 ride the HWDGE queues (scalar); w1/w3 stream on the SWDGE
        # gpsimd queue in strict need order.
        xt = xpool.tile([P, KD, C], F16)
        xT_r = xT_d.rearrange("(k p) c -> p k c", p=P)
        for q in range(KD // 2):
            ks = bass.ds(q * 2, 2)
            with tc.tile_wait_until(q * 0.002):
                nc.scalar.dma_start(xt[:, ks, :], xT_r[:, ks, :])

        w1s = w1pool.tile([P, KD, F], F16)
        w3s = w3pool.tile([P, KD, F], F16)
        w1_r = w1_d.rearrange("(k p) f -> p k f", p=P)
        w3_r = w3_d.rearrange("(k p) f -> p k f", p=P)
        for q in range(KD // 2):
            ks = bass.ds(q * 2, 2)
            with tc.tile_wait_until(q * 0.002):
                nc.gpsimd.dma_start(w1s[:, ks, 0:NB], w1_r[:, ks, 0:NB])
        for base_ms, ws, w_r, fb in (
            (0.020, w3s, w3_r, 0),
            (0.038, w1s, w1_r, 1),
            (0.056, w3s, w3_r, 1),
        ):
            fs = bass.ds(fb * NB, NB)
            for q in range(KD // 4):
                ks = bass.ds(q * 4, 4)
                with tc.tile_wait_until(base_ms + q * 0.004):
                    nc.gpsimd.dma_start(ws[:, ks, fs], w_r[:, ks, fs])

        w2s = w2pool.tile([P, KF, D], F16)
        w2_r = w2_d.rearrange("(k p) d -> p k d", p=P)
        for q in range(KF // 2):
            ks = bass.ds(q * 2, 2)
            with tc.tile_wait_until(0.072 + q * 0.008):
                nc.scalar.dma_start(w2s[:, ks, :], w2_r[:, ks, :])

        ssqa = stat.tile([P, NT], F32, name="ssqa")
        ssqb = stat.tile([P, NT], F32, name="ssqb")
        ssq = stat.tile([P, NT], F32, name="ssq")
        std = stat.tile([P, NT], F32, name="std")
        rstd = stat.tile([P, NT], F32, name="rstd")
        h = {
            t: hpool.tile([P, F], F16, tag=f"h{t}", name=f"h{t}")
            for t in range(NT)
        }
        ht = {
            t: htpool.tile([P, KF, P], F16, tag=f"ht{t}", name=f"ht{t}")
            for t in range(NT)
        }
        s = {}
        KQ = 4  # k-chunks per sweep quarter

        # ===== phase A: four sweeps (w1 fb0, w3 fb0, w1 fb1, w3 fb1).
        # Quarter-k-outer with t inner: weights are consumed progressively
        # (so DMA pacing works) AND each tile's epilogue lands right after
        # its last k-quarter, overlapping the next sweep's matmuls.
        def sweep(ws, fb, epilogue):
            ps = {}
            for t in range(NT):
                ps[t] = ps_a.tile(
                    [P, NB], F32, tag=f"pa{t}", name=f"ps_{fb}_{t}"
                )
            fs = bass.ds(fb * NB, NB)
            for kq in range(KD // KQ):
                for t in range(NT):
                    for k in range(kq * KQ, (kq + 1) * KQ):
                        nc.tensor.matmul(
                            ps[t][:],
                            xt[:, k, t * P : t * P + P],
                            ws[:, k, fs],
                            start=(k == 0),
                            stop=(k == KD - 1),
                        )
                    if kq == KD // KQ - 1:
                        epilogue(t, ps[t])

        def ep_silu1(t, ps):
            s[t] = spool.tile([P, NB], F32, tag=f"s{t}", name=f"s0_{t}")
            nc.scalar.activation(s[t][:], ps[:], ACTF.Silu)

        def ep_h0(t, ps):
            nc.vector.tensor_mul(h[t][:, 0:NB], s[t][:], ps[:])
            hsq = qpool.tile([P, NB], F16, tag="hsq", name=f"hsq0_{t}")
            nc.scalar.activation(
                hsq[:], h[t][:, 0:NB], ACTF.Square, accum_out=ssqa[:, t : t + 1]
            )
            nc.scalar.dma_start_transpose(
                ht[t][:, 0 : KF // 2, :], h[t][:, 0:NB]
            )

        def ep_silu2(t, ps):
            sn = spool.tile([P, NB], F32, tag=f"s{t}", name=f"s1_{t}")
            nc.scalar.activation(sn[:], ps[:], ACTF.Silu)
            s[t] = sn

        def ep_h1(t, ps):
            nc.vector.tensor_mul(h[t][:, NB : 2 * NB], s[t][:], ps[:])
            hsq = qpool.tile([P, NB], F16, tag="hsq", name=f"hsq1_{t}")
            nc.scalar.activation(
                hsq[:],
                h[t][:, NB : 2 * NB],
                ACTF.Square,
                accum_out=ssqb[:, t : t + 1],
            )
            nc.scalar.dma_start_transpose(
                ht[t][:, KF // 2 : KF, :], h[t][:, NB : 2 * NB]
            )
            nc.vector.tensor_add(
                ssq[:, t : t + 1], ssqa[:, t : t + 1], ssqb[:, t : t + 1]
            )
            nc.scalar.activation(
                std[:, t : t + 1],
                ssq[:, t : t + 1],
                ACTF.Sqrt,
                bias=eps_t[:],
                scale=1.0 / F,
            )
            nc.vector.reciprocal(rstd[:, t : t + 1], std[:, t : t + 1])

        sweep(w1s, 0, ep_silu1)
        sweep(w3s, 0, ep_h0)
        sweep(w1s, 1, ep_silu2)
        sweep(w3s, 1, ep_h1)

        # ===== phase C: out = hT.T @ w2T, scaled by rstd.  db outer so the
        # w2 stream is consumed progressively; out DMAs per (t, db) chunk.
        for db in range(DB):
            dsl = bass.ds(db * NB, NB)
            for t in range(NT):
                pso = ps_o.tile([P, NB], F32, tag="po")
                for fc in range(KF):
                    nc.tensor.matmul(
                        pso[:],
                        ht[t][:, fc, :],
                        w2s[:, fc, dsl],
                        start=(fc == 0),
                        stop=(fc == KF - 1),
                    )
                ob = opool.tile([P, NB], F16, tag="ob", name=f"ob{t}_{db}")
                nc.vector.tensor_scalar_mul(ob[:], pso[:], rstd[:, t : t + 1])
                nc.sync.dma_start(out_d[t * P : (t + 1) * P, dsl], ob[:])

    nc.compile()
    return nc


def _get_program(C: int):
    if C not in _PROGRAM_CACHE:
        _PROGRAM_CACHE[C] = _build_program(C)
    return _PROGRAM_CACHE[C]


def kernel(x, w1, w2, w3, mid_w, num_tokens_per_expert):
    global LAST_RESULTS
    x = np.ascontiguousarray(np.asarray(x, dtype=np.float32))
    w1 = np.asarray(w1, dtype=np.float32)
    w2 = np.asarray(w2, dtype=np.float32)
    w3 = np.asarray(w3, dtype=np.float32)
    mid_w = np.asarray(mid_w, dtype=np.float32)
    counts = np.asarray(num_tokens_per_expert).astype(np.int64)

    T_, D_ = x.shape
    E_, F_, _ = w1.shape
    Ccap = (T_ // E_) * 3 // 2  # reference static capacity (768)
    ends = np.cumsum(counts)
    starts = ends - counts
    eff = np.minimum(np.maximum(counts, 0), Ccap)  # rows actually computed

    C = int(max(P, -(-int(eff.max()) // P) * P))  # pad to token-tile multiple
    nc = _get_program(C)

    in_maps = []
    for e in range(E_):
        cnt = int(eff[e])
        s = int(starts[e])
        xg = np.zeros((C, D_), np.float32)
        if cnt > 0:
            rows = np.clip(s + np.arange(cnt), 0, T_ - 1)
            xg[:cnt] = x[rows]
        in_maps.append(
            {
                "xT": np.ascontiguousarray(xg.T).astype(np.float16),
                "w1t": np.ascontiguousarray(w1[e].T).astype(np.float16),
                "w3t": np.ascontiguousarray(w3[e].T).astype(np.float16),
                "w2t": np.ascontiguousarray((w2[e] * mid_w[None, :]).T).astype(
                    np.float16
                ),
            }
        )

    LAST_RESULTS = _run(nc, in_maps)
    outs = [LAST_RESULTS[e]["out"] for e in range(E_)]

    # scatter back to flat token order, mirroring the reference's clamping
    tok = np.arange(T_)
    eid = np.clip(np.searchsorted(ends, tok, side="right"), 0, E_ - 1)
    pos = tok - starts[eid]
    idx = np.minimum(pos, Ccap - 1)
    valid = (idx >= 0) & (idx < eff[eid])
    idx_safe = np.clip(idx, 0, C - 1)
    stacked = np.stack(outs, axis=0)  # [E, C, D]
    result = stacked[eid, idx_safe].astype(np.float32)
    result[~valid] = 0.0
    return result


# revision 15
# speedup vs baseline: 1.0794x; 1.0794x over previous
"""Trainium2 Bass kernel: grouped-experts SwiGLU MLP with mid-RMSNorm.

Expert-parallel across 8 NeuronCores: core e computes expert e's token
block (tokens are pre-sorted by expert).  Host gathers each expert's
rows into a zero-padded [C, D] buffer, ships transposed activations and
weights, and scatters the per-core outputs back to flat token order.

Per-core math (fp16 operands, fp32 PSUM accumulation):
    h1 = x @ w1^T ; h3 = x @ w3^T          # [C, F]
    h  = silu(h1) * h3
    h  = h * rsqrt(mean(h^2) + eps)        # RMSNorm (scale folded to out)
    out = (h * mid_w) @ w2^T               # mid_w folded into w2 on host

Schedule (all weights SBUF-resident, streamed in once):
  fb0 w1-sweep k-outer (paces the initial weight/x DMAs), then t-outer
  sweeps for fb0 w3, fb1 w1, fb1 w3.  h tiles are transposed via the
  DMA xbar (dma_start_transpose) so the PE only runs the 480 matmuls.
  Phase C (out = hT.T @ w2T, scaled by rstd) runs per-tile at the end.
"""

import sys

sys.path.insert(0, "/opt/trn_rl_repo")

import numpy as np
from contextlib import ExitStack

import os

import concourse.bass as bass
import concourse.tile as tile
from concourse import bacc, mybir

P = 128
T = 4096
D = 2048
F = 1024
E = 8
NB = 512  # matmul moving-dim block (one PSUM bank of fp32)
EPS = 1e-6
F32 = mybir.dt.float32
F16 = mybir.dt.float16
ACTF = mybir.ActivationFunctionType

_PROGRAM_CACHE: dict[int, object] = {}
LAST_RESULTS = None  # test harness reads per-core outputs from here


def _run(nc, in_maps):
    """Execute the compiled program on the 8 axon-tunneled cores.

    If KERNEL_NTFF_DIR is set, wrap the execute in the axon NTFF profile
    hook so device profiles land there (test harness use only).
    """
    from concourse import bass2jax

    ntff_dir = os.environ.get("KERNEL_NTFF_DIR")
    if ntff_dir:
        if "/root/.axon_site" not in sys.path:
            sys.path.insert(0, "/root/.axon_site")
        from trn_agent_boot.trn_boot import _ntff_profile_via_ctypes

        hook = _ntff_profile_via_ctypes("/opt/axon/libaxon_pjrt.so")
        ids = [
            int(x) for x in os.environ.get("KERNEL_NTFF_CORES", "0").split(",")
        ]
        if hook is not None:
            with hook(ntff_dir, ids):
                return bass2jax.run_bass_via_pjrt(nc, in_maps, n_cores=len(in_maps))
    return bass2jax.run_bass_via_pjrt(nc, in_maps, n_cores=len(in_maps))


def _build_program(C: int):
    """Build + compile the single-core SPMD program for C padded rows."""
    NT = C // P  # token tiles per core (<= 6: PSUM holds NT + 2 banks)
    KD = D // P  # 16 contraction chunks for mm1
    KF = F // P  # 8 contraction chunks for mm2
    FB = F // NB  # 2 f-blocks
    DB = D // NB  # 4 d-blocks

    nc = bacc.Bacc(
        "TRN2",
        target_bir_lowering=False,
        debug=False,
        enable_asserts=False,
        num_devices=E,
    )
    xT_d = nc.dram_tensor("xT", [D, C], F16, kind="ExternalInput").ap()
    w1_d = nc.dram_tensor("w1t", [D, F], F16, kind="ExternalInput").ap()
    w3_d = nc.dram_tensor("w3t", [D, F], F16, kind="ExternalInput").ap()
    w2_d = nc.dram_tensor("w2t", [F, D], F16, kind="ExternalInput").ap()
    out_d = nc.dram_tensor("out", [C, D], F16, kind="ExternalOutput").ap()

    with tile.TileContext(nc) as tc, ExitStack() as ctx:
        singles = ctx.enter_context(tc.tile_pool(name="singles", bufs=1))
        xpool = ctx.enter_context(tc.tile_pool(name="x", bufs=1))
        w1pool = ctx.enter_context(tc.tile_pool(name="w1", bufs=1))
        w3pool = ctx.enter_context(tc.tile_pool(name="w3", bufs=1))
        w2pool = ctx.enter_context(tc.tile_pool(name="w2", bufs=1))
        hpool = ctx.enter_context(tc.tile_pool(name="h", bufs=1))
        htpool = ctx.enter_context(tc.tile_pool(name="ht", bufs=1))
        spool = ctx.enter_context(tc.tile_pool(name="s", bufs=1))
        qpool = ctx.enter_context(tc.tile_pool(name="sq", bufs=2))
        opool = ctx.enter_context(tc.tile_pool(name="o", bufs=2))
        stat = ctx.enter_context(tc.tile_pool(name="stat", bufs=1))
        ps_a = ctx.enter_context(tc.tile_pool(name="psa", bufs=1, space="PSUM"))
        ps_o = ctx.enter_context(tc.tile_pool(name="pso", bufs=2, space="PSUM"))

        eps_t = singles.tile([P, 1], F32, name="epsT")
        nc.gpsimd.memset(eps_t[:], EPS)

        # ---- input DMAs, issued upfront but paced with scheduler time
        # floors so the instantaneous HBM rate stays low: saturating DMA
        # roughly halves the PE matmul issue rate (SBUF port contention),
        # so every stream arrives just-in-time for its sweep.
        # x + w2 ride the HWDGE queues (scalar); w1/w3 stream on the SWDGE
        # gpsimd queue in strict need order.
        xt = xpool.tile([P, KD, C], F16)
        xT_r = xT_d.rearrange("(k p) c -> p k c", p=P)
        for q in range(KD // 2):
            ks = bass.ds(q * 2, 2)
            nc.scalar.dma_start(xt[:, ks, :], xT_r[:, ks, :])

        w1s = w1pool.tile([P, KD, F], F16)
        w3s = w3pool.tile([P, KD, F], F16)
        w1_r = w1_d.rearrange("(k p) f -> p k f", p=P)
        w3_r = w3_d.rearrange("(k p) f -> p k f", p=P)
        # fb0 weights stream immediately (gpsimd SWDGE, strict need order);
        # fb1 weights and w2 get scheduler time floors so they don't steal
        # HBM bandwidth from the first sweeps.
        for ws, w_r in ((w1s, w1_r), (w3s, w3_r)):
            for q in range(KD // 2):
                ks = bass.ds(q * 2, 2)
                nc.gpsimd.dma_start(ws[:, ks, 0:NB], w_r[:, ks, 0:NB])
        for base_ms, ws, w_r in ((0.028, w1s, w1_r), (0.040, w3s, w3_r)):
            for q in range(KD // 4):
                ks = bass.ds(q * 4, 4)
                with tc.tile_wait_until(base_ms + q * 0.003):
                    nc.gpsimd.dma_start(
                        ws[:, ks, NB : 2 * NB], w_r[:, ks, NB : 2 * NB]
                    )

        w2s = w2pool.tile([P, KF, D], F16)
        w2_r = w2_d.rearrange("(k p) d -> p k d", p=P)
        for q in range(KF // 2):
            ks = bass.ds(q * 2, 2)
            with tc.tile_wait_until(0.052 + q * 0.005):
                nc.scalar.dma_start(w2s[:, ks, :], w2_r[:, ks, :])

        ssqa = stat.tile([P, NT], F32, name="ssqa")
        ssqb = stat.tile([P, NT], F32, name="ssqb")
        ssq = stat.tile([P, NT], F32, name="ssq")
        std = stat.tile([P, NT], F32, name="std")
        rstd = stat.tile([P, NT], F32, name="rstd")
        h = {
            t: hpool.tile([P, F], F16, tag=f"h{t}", name=f"h{t}")
            for t in range(NT)
        }
        ht = {
            t: htpool.tile([P, KF, P], F16, tag=f"ht{t}", name=f"ht{t}")
            for t in range(NT)
        }
        s = {}
        KQ = 4  # k-chunks per sweep quarter

        # ===== phase A: four sweeps (w1 fb0, w3 fb0, w1 fb1, w3 fb1).
        # Sweep 1 is k-outer so the PE consumes the just-arriving x/w1
        # chunks progressively; later sweeps have resident weights and run
        # t-outer so each tile's epilogue overlaps the next tile's matmuls.
        def sweep(ws, fb, epilogue, k_outer=False):
            ps = {}
            for t in range(NT):
                ps[t] = ps_a.tile(
                    [P, NB], F32, tag=f"pa{t}", name=f"ps_{fb}_{t}"
                )
            fs = bass.ds(fb * NB, NB)
            if k_outer:
                for k in range(KD):
                    for t in range(NT):
                        nc.tensor.matmul(
                            ps[t][:],
                            xt[:, k, t * P : t * P + P],
                            ws[:, k, fs],
                            start=(k == 0),
                            stop=(k == KD - 1),
                        )
                for t in range(NT):
                    epilogue(t, ps[t])
            else:
                for t in range(NT):
                    for k in range(KD):
                        nc.tensor.matmul(
                            ps[t][:],
                            xt[:, k, t * P : t * P + P],
                            ws[:, k, fs],
                            start=(k == 0),
                            stop=(k == KD - 1),
                        )
                    epilogue(t, ps[t])

        def ep_silu1(t, ps):
            s[t] = spool.tile([P, NB], F32, tag=f"s{t}", name=f"s0_{t}")
            nc.scalar.activation(s[t][:], ps[:], ACTF.Silu)

        def ep_h0(t, ps):
            nc.vector.tensor_mul(h[t][:, 0:NB], s[t][:], ps[:])
            hsq = qpool.tile([P, NB], F16, tag="hsq", name=f"hsq0_{t}")
            nc.scalar.activation(
                hsq[:], h[t][:, 0:NB], ACTF.Square, accum_out=ssqa[:, t : t + 1]
            )
            nc.scalar.dma_start_transpose(
                ht[t][:, 0 : KF // 2, :], h[t][:, 0:NB]
            )

        def ep_silu2(t, ps):
            sn = spool.tile([P, NB], F32, tag=f"s{t}", name=f"s1_{t}")
            nc.scalar.activation(sn[:], ps[:], ACTF.Silu)
            s[t] = sn

        def ep_h1(t, ps):
            nc.vector.tensor_mul(h[t][:, NB : 2 * NB], s[t][:], ps[:])
            hsq = qpool.tile([P, NB], F16, tag="hsq", name=f"hsq1_{t}")
            nc.scalar.activation(
                hsq[:],
                h[t][:, NB : 2 * NB],
                ACTF.Square,
                accum_out=ssqb[:, t : t + 1],
            )
            nc.scalar.dma_start_transpose(
                ht[t][:, KF // 2 : KF, :], h[t][:, NB : 2 * NB]
            )
            nc.vector.tensor_add(
                ssq[:, t : t + 1], ssqa[:, t : t + 1], ssqb[:, t : t + 1]
            )
            nc.scalar.activation(
                std[:, t : t + 1],
                ssq[:, t : t + 1],
                ACTF.Sqrt,
                bias=eps_t[:],
                scale=1.0 / F,
            )
            nc.vector.reciprocal(rstd[:, t : t + 1], std[:, t : t + 1])

        sweep(w1s, 0, ep_silu1, k_outer=True)
        sweep(w3s, 0, ep_h0)
        sweep(w1s, 1, ep_silu2)
        sweep(w3s, 1, ep_h1)

        # ===== phase C: out = hT.T @ w2T, scaled by rstd.  db outer so the
        # w2 stream is consumed progressively; out DMAs per (t, db) chunk.
        for db in range(DB):
            dsl = bass.ds(db * NB, NB)
            for t in range(NT):
                pso = ps_o.tile([P, NB], F32, tag="po")
                for fc in range(KF):
                    nc.tensor.matmul(
                        pso[:],
                        ht[t][:, fc, :],
                        w2s[:, fc, dsl],
                        start=(fc == 0),
                        stop=(fc == KF - 1),
                    )
                ob = opool.tile([P, NB], F16, tag="ob", name=f"ob{t}_{db}")
                nc.vector.tensor_scalar_mul(ob[:], pso[:], rstd[:, t : t + 1])
                nc.sync.dma_start(out_d[t * P : (t + 1) * P, dsl], ob[:])

    nc.compile()
    return nc


def _get_program(C: int):
    if C not in _PROGRAM_CACHE:
        _PROGRAM_CACHE[C] = _build_program(C)
    return _PROGRAM_CACHE[C]


def kernel(x, w1, w2, w3, mid_w, num_tokens_per_expert):
    global LAST_RESULTS
    x = np.ascontiguousarray(np.asarray(x, dtype=np.float32))
    w1 = np.asarray(w1, dtype=np.float32)
    w2 = np.asarray(w2, dtype=np.float32)
    w3 = np.asarray(w3, dtype=np.float32)
    mid_w = np.asarray(mid_w, dtype=np.float32)
    counts = np.asarray(num_tokens_per_expert).astype(np.int64)

    T_, D_ = x.shape
    E_, F_, _ = w1.shape
    Ccap = (T_ // E_) * 3 // 2  # reference static capacity (768)
    ends = np.cumsum(counts)
    starts = ends - counts
    eff = np.minimum(np.maximum(counts, 0), Ccap)  # rows actually computed

    C = int(max(P, -(-int(eff.max()) // P) * P))  # pad to token-tile multiple
    nc = _get_program(C)

    in_maps = []
    for e in range(E_):
        cnt = int(eff[e])
        s = int(starts[e])
        xg = np.zeros((C, D_), np.float32)
        if cnt > 0:
            rows = np.clip(s + np.arange(cnt), 0, T_ - 1)
            xg[:cnt] = x[rows]
        in_maps.append(
            {
                "xT": np.ascontiguousarray(xg.T).astype(np.float16),
                "w1t": np.ascontiguousarray(w1[e].T).astype(np.float16),
                "w3t": np.ascontiguousarray(w3[e].T).astype(np.float16),
                "w2t": np.ascontiguousarray((w2[e] * mid_w[None, :]).T).astype(
                    np.float16
                ),
            }
        )

    LAST_RESULTS = _run(nc, in_maps)
    outs = [LAST_RESULTS[e]["out"] for e in range(E_)]

    # scatter back to flat token order, mirroring the reference's clamping
    tok = np.arange(T_)
    eid = np.clip(np.searchsorted(ends, tok, side="right"), 0, E_ - 1)
    pos = tok - starts[eid]
    idx = np.minimum(pos, Ccap - 1)
    valid = (idx >= 0) & (idx < eff[eid])
    idx_safe = np.clip(idx, 0, C - 1)
    stacked = np.stack(outs, axis=0)  # [E, C, D]
    result = stacked[eid, idx_safe].astype(np.float32)
    result[~valid] = 0.0
    return result


# revision 16
# speedup vs baseline: 1.2206x; 1.1309x over previous
"""Trainium2 Bass kernel: grouped-experts SwiGLU MLP with mid-RMSNorm.

Expert-parallel across 8 NeuronCores: core e computes expert e's token
block (tokens are pre-sorted by expert).  Host gathers each expert's
rows into a zero-padded [C, D] buffer, ships transposed activations and
weights, and scatters the per-core outputs back to flat token order.

Per-core math (fp16 operands, fp32 PSUM accumulation):
    h1 = x @ w1^T ; h3 = x @ w3^T          # [C, F]
    h  = silu(h1) * h3
    h  = h * rsqrt(mean(h^2) + eps)        # RMSNorm (scale folded to out)
    out = (h * mid_w) @ w2^T               # mid_w folded into w2 on host

Schedule (all weights SBUF-resident, streamed in once):
  fb0 w1-sweep k-outer (paces the initial weight/x DMAs), then t-outer
  sweeps for fb0 w3, fb1 w1, fb1 w3.  h tiles are transposed via the
  DMA xbar (dma_start_transpose) so the PE only runs the 480 matmuls.
  Phase C (out = hT.T @ w2T, scaled by rstd) runs per-tile at the end.
"""

import sys

sys.path.insert(0, "/opt/trn_rl_repo")

import numpy as np
from contextlib import ExitStack

import os

import concourse.bass as bass
import concourse.tile as tile
from concourse import bacc, mybir

P = 128
T = 4096
D = 2048
F = 1024
E = 8
NB = 512  # matmul moving-dim block (one PSUM bank of fp32)
EPS = 1e-6
F32 = mybir.dt.float32
F16 = mybir.dt.float16
ACTF = mybir.ActivationFunctionType

_PROGRAM_CACHE: dict[int, object] = {}
LAST_RESULTS = None  # test harness reads per-core outputs from here


def _run(nc, in_maps):
    """Execute the compiled program on the 8 axon-tunneled cores.

    If KERNEL_NTFF_DIR is set, wrap the execute in the axon NTFF profile
    hook so device profiles land there (test harness use only).
    """
    from concourse import bass2jax

    ntff_dir = os.environ.get("KERNEL_NTFF_DIR")
    if ntff_dir:
        if "/root/.axon_site" not in sys.path:
            sys.path.insert(0, "/root/.axon_site")
        from trn_agent_boot.trn_boot import _ntff_profile_via_ctypes

        hook = _ntff_profile_via_ctypes("/opt/axon/libaxon_pjrt.so")
        ids = [
            int(x) for x in os.environ.get("KERNEL_NTFF_CORES", "0").split(",")
        ]
        if hook is not None:
            with hook(ntff_dir, ids):
                return bass2jax.run_bass_via_pjrt(nc, in_maps, n_cores=len(in_maps))
    return bass2jax.run_bass_via_pjrt(nc, in_maps, n_cores=len(in_maps))


def _build_program(C: int):
    """Build + compile the single-core SPMD program for C padded rows."""
    NT = C // P  # token tiles per core (<= 6: PSUM holds NT + 2 banks)
    KD = D // P  # 16 contraction chunks for mm1
    KF = F // P  # 8 contraction chunks for mm2
    FB = F // NB  # 2 f-blocks
    DB = D // NB  # 4 d-blocks

    nc = bacc.Bacc(
        "TRN2",
        target_bir_lowering=False,
        debug=False,
        enable_asserts=False,
        num_devices=E,
    )
    xT_d = nc.dram_tensor("xT", [D, C], F16, kind="ExternalInput").ap()
    w1_d = nc.dram_tensor("w1t", [D, F], F16, kind="ExternalInput").ap()
    w3_d = nc.dram_tensor("w3t", [D, F], F16, kind="ExternalInput").ap()
    w2_d = nc.dram_tensor("w2t", [F, D], F16, kind="ExternalInput").ap()
    out_d = nc.dram_tensor("out", [C, D], F16, kind="ExternalOutput").ap()

    with tile.TileContext(nc) as tc, ExitStack() as ctx:
        singles = ctx.enter_context(tc.tile_pool(name="singles", bufs=1))
        xpool = ctx.enter_context(tc.tile_pool(name="x", bufs=1))
        w1pool = ctx.enter_context(tc.tile_pool(name="w1", bufs=1))
        w3pool = ctx.enter_context(tc.tile_pool(name="w3", bufs=1))
        w2pool = ctx.enter_context(tc.tile_pool(name="w2", bufs=1))
        hpool = ctx.enter_context(tc.tile_pool(name="h", bufs=1))
        htpool = ctx.enter_context(tc.tile_pool(name="ht", bufs=1))
        spool = ctx.enter_context(tc.tile_pool(name="s", bufs=1))
        qpool = ctx.enter_context(tc.tile_pool(name="sq", bufs=2))
        opool = ctx.enter_context(tc.tile_pool(name="o", bufs=2))
        stat = ctx.enter_context(tc.tile_pool(name="stat", bufs=1))
        ps_a = ctx.enter_context(tc.tile_pool(name="psa", bufs=1, space="PSUM"))
        ps_o = ctx.enter_context(tc.tile_pool(name="pso", bufs=2, space="PSUM"))

        eps_t = singles.tile([P, 1], F32, name="epsT")
        nc.gpsimd.memset(eps_t[:], EPS)

        # ---- input DMAs, issued upfront but paced with scheduler time
        # floors so the instantaneous HBM rate stays low: saturating DMA
        # roughly halves the PE matmul issue rate (SBUF port contention),
        # so every stream arrives just-in-time for its sweep.
        # x + w2 ride the HWDGE queues (scalar); w1/w3 stream on the SWDGE
        # gpsimd queue in strict need order.
        xt = xpool.tile([P, KD, C], F16)
        xT_r = xT_d.rearrange("(k p) c -> p k c", p=P)
        for q in range(KD // 2):
            ks = bass.ds(q * 2, 2)
            nc.scalar.dma_start(xt[:, ks, :], xT_r[:, ks, :])

        w1s = w1pool.tile([P, KD, F], F16)
        w3s = w3pool.tile([P, KD, F], F16)
        w1_r = w1_d.rearrange("(k p) f -> p k f", p=P)
        w3_r = w3_d.rearrange("(k p) f -> p k f", p=P)
        # all weight streams go out immediately, in strict need order on the
        # gpsimd SWDGE queue (which issues back-to-back without ring
        # stalls): w1 fb0, w3 fb0, w1 fb1, w3 fb1.
        for ws, w_r in ((w1s, w1_r), (w3s, w3_r)):
            for q in range(KD // 2):
                ks = bass.ds(q * 2, 2)
                nc.gpsimd.dma_start(ws[:, ks, 0:NB], w_r[:, ks, 0:NB])
        for ws, w_r in ((w1s, w1_r), (w3s, w3_r)):
            for q in range(KD // 4):
                ks = bass.ds(q * 4, 4)
                nc.gpsimd.dma_start(
                    ws[:, ks, NB : 2 * NB], w_r[:, ks, NB : 2 * NB]
                )

        w2s = w2pool.tile([P, KF, D], F16)
        w2_r = w2_d.rearrange("(k p) d -> p k d", p=P)
        for q in range(KF // 2):
            ks = bass.ds(q * 2, 2)
            nc.scalar.dma_start(w2s[:, ks, :], w2_r[:, ks, :])

        ssqa = stat.tile([P, NT], F32, name="ssqa")
        ssqb = stat.tile([P, NT], F32, name="ssqb")
        ssq = stat.tile([P, NT], F32, name="ssq")
        std = stat.tile([P, NT], F32, name="std")
        rstd = stat.tile([P, NT], F32, name="rstd")
        h = {
            t: hpool.tile([P, F], F16, tag=f"h{t}", name=f"h{t}")
            for t in range(NT)
        }
        ht = {
            t: htpool.tile([P, KF, P], F16, tag=f"ht{t}", name=f"ht{t}")
            for t in range(NT)
        }
        s = {}
        KQ = 4  # k-chunks per sweep quarter

        # ===== phase A: four sweeps (w1 fb0, w3 fb0, w1 fb1, w3 fb1).
        # Sweep 1 is k-outer so the PE consumes the just-arriving x/w1
        # chunks progressively; later sweeps have resident weights and run
        # t-outer so each tile's epilogue overlaps the next tile's matmuls.
        def sweep(ws, fb, epilogue, k_outer=False):
            ps = {}
            for t in range(NT):
                ps[t] = ps_a.tile(
                    [P, NB], F32, tag=f"pa{t}", name=f"ps_{fb}_{t}"
                )
            fs = bass.ds(fb * NB, NB)
            if k_outer:
                for k in range(KD):
                    for t in range(NT):
                        nc.tensor.matmul(
                            ps[t][:],
                            xt[:, k, t * P : t * P + P],
                            ws[:, k, fs],
                            start=(k == 0),
                            stop=(k == KD - 1),
                        )
                for t in range(NT):
                    epilogue(t, ps[t])
            else:
                for t in range(NT):
                    for k in range(KD):
                        nc.tensor.matmul(
                            ps[t][:],
                            xt[:, k, t * P : t * P + P],
                            ws[:, k, fs],
                            start=(k == 0),
                            stop=(k == KD - 1),
                        )
                    epilogue(t, ps[t])

        def ep_silu1(t, ps):
            s[t] = spool.tile([P, NB], F32, tag=f"s{t}", name=f"s0_{t}")
            nc.scalar.activation(s[t][:], ps[:], ACTF.Silu)

        def ep_h0(t, ps):
            nc.vector.tensor_mul(h[t][:, 0:NB], s[t][:], ps[:])
            hsq = qpool.tile([P, NB], F16, tag="hsq", name=f"hsq0_{t}")
            nc.scalar.activation(
                hsq[:], h[t][:, 0:NB], ACTF.Square, accum_out=ssqa[:, t : t + 1]
            )
            nc.scalar.dma_start_transpose(
                ht[t][:, 0 : KF // 2, :], h[t][:, 0:NB]
            )

        def ep_silu2(t, ps):
            sn = spool.tile([P, NB], F32, tag=f"s{t}", name=f"s1_{t}")
            nc.scalar.activation(sn[:], ps[:], ACTF.Silu)
            s[t] = sn

        def ep_h1(t, ps):
            nc.vector.tensor_mul(h[t][:, NB : 2 * NB], s[t][:], ps[:])
            hsq = qpool.tile([P, NB], F16, tag="hsq", name=f"hsq1_{t}")
            nc.scalar.activation(
                hsq[:],
                h[t][:, NB : 2 * NB],
                ACTF.Square,
                accum_out=ssqb[:, t : t + 1],
            )
            nc.scalar.dma_start_transpose(
                ht[t][:, KF // 2 : KF, :], h[t][:, NB : 2 * NB]
            )
            nc.vector.tensor_add(
                ssq[:, t : t + 1], ssqa[:, t : t + 1], ssqb[:, t : t + 1]
            )
            nc.scalar.activation(
                std[:, t : t + 1],
                ssq[:, t : t + 1],
                ACTF.Sqrt,
                bias=eps_t[:],
                scale=1.0 / F,
            )
            nc.vector.reciprocal(rstd[:, t : t + 1], std[:, t : t + 1])

        sweep(w1s, 0, ep_silu1, k_outer=True)
        sweep(w3s, 0, ep_h0)
        sweep(w1s, 1, ep_silu2)
        sweep(w3s, 1, ep_h1)

        # ===== phase C: out = hT.T @ w2T, scaled by rstd.  db outer so the
        # w2 stream is consumed progressively; out DMAs per (t, db) chunk.
        for db in range(DB):
            dsl = bass.ds(db * NB, NB)
            for t in range(NT):
                pso = ps_o.tile([P, NB], F32, tag="po")
                for fc in range(KF):
                    nc.tensor.matmul(
                        pso[:],
                        ht[t][:, fc, :],
                        w2s[:, fc, dsl],
                        start=(fc == 0),
                        stop=(fc == KF - 1),
                    )
                ob = opool.tile([P, NB], F16, tag="ob", name=f"ob{t}_{db}")
                nc.vector.tensor_scalar_mul(ob[:], pso[:], rstd[:, t : t + 1])
                nc.sync.dma_start(out_d[t * P : (t + 1) * P, dsl], ob[:])

    nc.compile()
    return nc


def _get_program(C: int):
    if C not in _PROGRAM_CACHE:
        _PROGRAM_CACHE[C] = _build_program(C)
    return _PROGRAM_CACHE[C]


def kernel(x, w1, w2, w3, mid_w, num_tokens_per_expert):
    global LAST_RESULTS
    x = np.ascontiguousarray(np.asarray(x, dtype=np.float32))
    w1 = np.asarray(w1, dtype=np.float32)
    w2 = np.asarray(w2, dtype=np.float32)
    w3 = np.asarray(w3, dtype=np.float32)
    mid_w = np.asarray(mid_w, dtype=np.float32)
    counts = np.asarray(num_tokens_per_expert).astype(np.int64)

    T_, D_ = x.shape
    E_, F_, _ = w1.shape
    Ccap = (T_ // E_) * 3 // 2  # reference static capacity (768)
    ends = np.cumsum(counts)
    starts = ends - counts
    eff = np.minimum(np.maximum(counts, 0), Ccap)  # rows actually computed

    C = int(max(P, -(-int(eff.max()) // P) * P))  # pad to token-tile multiple
    nc = _get_program(C)

    in_maps = []
    for e in range(E_):
        cnt = int(eff[e])
        s = int(starts[e])
        xg = np.zeros((C, D_), np.float32)
        if cnt > 0:
            rows = np.clip(s + np.arange(cnt), 0, T_ - 1)
            xg[:cnt] = x[rows]
        in_maps.append(
            {
                "xT": np.ascontiguousarray(xg.T).astype(np.float16),
                "w1t": np.ascontiguousarray(w1[e].T).astype(np.float16),
                "w3t": np.ascontiguousarray(w3[e].T).astype(np.float16),
                "w2t": np.ascontiguousarray((w2[e] * mid_w[None, :]).T).astype(
                    np.float16
                ),
            }
        )

    LAST_RESULTS = _run(nc, in_maps)
    outs = [LAST_RESULTS[e]["out"] for e in range(E_)]

    # scatter back to flat token order, mirroring the reference's clamping
    tok = np.arange(T_)
    eid = np.clip(np.searchsorted(ends, tok, side="right"), 0, E_ - 1)
    pos = tok - starts[eid]
    idx = np.minimum(pos, Ccap - 1)
    valid = (idx >= 0) & (idx < eff[eid])
    idx_safe = np.clip(idx, 0, C - 1)
    stacked = np.stack(outs, axis=0)  # [E, C, D]
    result = stacked[eid, idx_safe].astype(np.float32)
    result[~valid] = 0.0
    return result
